# revision 1
# baseline (speedup 1.0000x reference)
"""CLRNet loss kernel for Trainium2 (8 NeuronCores, data-parallel over batch).

Contract: kernel(predictions [3,512,192,78] f32, targets [512,4,78] f32,
seg_loss scalar f32) -> scalar f32 (full loss). Sharding: batch axis split
8 ways; each core computes partial (cls, reg, iou) sums over its 64 samples
x 3 stages; host combines.

Key algebra used on-device:
  ovr.sum  = 30*T - S_px,  union.sum = 30*T + S_px, where
  S_px[p,l] = sum_n mask[l,n] * |pred_px[p,n] - t_xs[l,n]| = 799 * S'
  S'[p,l]   = sum_n |max(t'[l,n] - p_hat[p,n], -1)| - n_invalid[l]
  (valid entries of t' = t/799 are in [0,1) so the clamp only hits invalid
   entries (t' ~ -125), each contributing exactly 1.0, subtracted exactly.)
The assignment distance is dist_px = S_px / (T + 1e-9); since scores are
ratios x/max(x), the 799 scale cancels and S' is used directly.
"""

import numpy as np

import concourse.bacc as bacc
import concourse.mybir as mybir
from concourse.tile import TileContext
from concourse.bass_utils import run_bass_kernel_spmd

F32 = mybir.dt.float32
Alu = mybir.AluOpType
AF = mybir.ActivationFunctionType
AX = mybir.AxisListType

STAGES, B, P, D = 3, 512, 192, 78
NPTS = 72
L = 4
NCORES = 8
BS = B // NCORES            # 64 batch samples per core
ROWS = STAGES * BS          # 192 virtual samples per core
IMG_W, IMG_H = 800.0, 320.0
N_STRIPS = float(NPTS - 1)
W_SCALE = IMG_W - 1.0       # 799
BIG = 1.0e30
PC = 64                     # prior chunk for the heavy S pipeline


def _build_block(nc, tc, pool, vpool, psum_pool, pd_small_dram, pd_xs_dram,
                 tg_dram, acc, s):
    """Emit one block of `s` samples (s <= 128 partitions)."""
    V, G, A, T = nc.vector, nc.gpsimd, nc.scalar, nc.tensor

    # ---------------- loads ----------------
    pd_s = pool.tile([s, P, 6], F32, tag="pd_small")
    nc.sync.dma_start(pd_s[:], pd_small_dram)
    tg = pool.tile([s, L, D], F32, tag="tg")
    r0 = 0
    for src in tg_dram:
        n = src.shape[0]
        nc.sync.dma_start(tg[r0:r0 + n], src)
        r0 += n
    pd_xc = []
    for pc in range(P // PC):
        t_ = pool.tile([s, PC, NPTS], F32, tag="pd_xc", bufs=4, name=f"pd_xc{pc}")
        nc.sync.dma_start(t_[:], pd_xs_dram[:, pc * PC:(pc + 1) * PC])
        pd_xc.append(t_)

    # big reusable [s, L, P] buffers
    bufs = [pool.tile([s, L, P], F32, tag=f"big{i}", name=f"big{i}") for i in range(8)]
    b0, b1, b2, b3, b4, b5, b6, b7 = bufs

    # ---------------- target-derived small tensors ----------------
    tp = pool.tile([s, L, NPTS], F32, tag="tp")          # t' = t_xs / 799
    V.tensor_scalar(tp[:], tg[:, :, 6:D], 1.0 / W_SCALE, None, op0=Alu.mult)
    validf = pool.tile([s, L], F32, tag="validf")
    V.tensor_scalar(validf[:], tg[:, :, 1], 1.0, None, op0=Alu.is_equal)
    invm = pool.tile([s, L, NPTS], F32, tag="invm")
    V.tensor_scalar(invm[:], tp[:], 0.0, None, op0=Alu.is_lt)
    n_inv = pool.tile([s, L], F32, tag="n_inv")
    V.tensor_reduce(out=n_inv[:], in_=invm[:], axis=AX.X, op=Alu.add)
    t_len = pool.tile([s, L], F32, tag="t_len")
    V.tensor_scalar(t_len[:], n_inv[:], -1.0, 72.0, op0=Alu.mult, op1=Alu.add)
    rec_tlen = pool.tile([s, L], F32, tag="rec_tlen")
    V.tensor_scalar(rec_tlen[:], t_len[:], 1e-9, None, op0=Alu.add)
    V.reciprocal(rec_tlen[:], rec_tlen[:])
    c1 = pool.tile([s, L], F32, tag="c1")                # 30*T/799
    V.tensor_scalar(c1[:], t_len[:], 30.0 / W_SCALE, None, op0=Alu.mult)
    c1eps = pool.tile([s, L], F32, tag="c1eps")
    V.tensor_scalar(c1eps[:], c1[:], 1e-9 / W_SCALE, None, op0=Alu.add)
    t_y = pool.tile([s, L], F32, tag="t_y")    # -(IMG_H-1)*tg2 (bias for dy)
    V.tensor_scalar(t_y[:], tg[:, :, 2], -(IMG_H - 1.0), None, op0=Alu.mult)
    ntx = pool.tile([s, L], F32, tag="ntx")    # -tg3 (bias for dx)
    V.tensor_scalar(ntx[:], tg[:, :, 3], -1.0, None, op0=Alu.mult)
    nth = pool.tile([s, L], F32, tag="nth")    # -tg4 (bias for th)
    V.tensor_scalar(nth[:], tg[:, :, 4], -1.0, None, op0=Alu.mult)
    # tstart = round(tg2*71) (exact integer recovery), tsum = tg5 + tstart
    tsum = pool.tile([s, L], F32, tag="tsum")
    ts0 = pool.tile([s, L], F32, tag="ts0")
    V.tensor_scalar(ts0[:], tg[:, :, 2], N_STRIPS, None, op0=Alu.mult)
    tsi = pool.tile([s, L], mybir.dt.int32, tag="tsi")
    V.tensor_copy(tsi[:], ts0[:])
    V.tensor_copy(tsum[:], tsi[:])          # round-to-nearest-even cast
    V.tensor_tensor(tsum[:], tsum[:], tg[:, :, 5], op=Alu.add)
    # reg targets g3[s,l,c]: [tg2*71, tg3, tg4*180]
    g3 = pool.tile([s, L, 3], F32, tag="g3")
    V.tensor_scalar(g3[:, :, 0], tg[:, :, 2], N_STRIPS, None, op0=Alu.mult)
    V.tensor_copy(g3[:, :, 1], tg[:, :, 3])
    V.tensor_scalar(g3[:, :, 2], tg[:, :, 4], 180.0, None, op0=Alu.mult)
    neg_pen = pool.tile([s, L], F32, tag="neg_pen")      # 0 valid / -BIG invalid
    V.tensor_scalar(neg_pen[:], validf[:], BIG, BIG, op0=Alu.mult, op1=Alu.subtract)
    iota4 = pool.tile([s, L], F32, tag="iota4")
    for j in range(L):
        V.memset(iota4[:, j:j + 1], float(j))
    eps12 = pool.tile([s, 1], F32, tag="eps12")
    V.memset(eps12[:], 1e-12)
    eps8 = pool.tile([s, 1], F32, tag="eps8")
    V.memset(eps8[:], 1e-8)
    c101 = pool.tile([s, 1], F32, tag="c101")
    V.memset(c101[:], 1.01)
    neg1t = pool.tile([s, 1], F32, tag="neg1t")
    V.memset(neg1t[:], -1.0)

    def bl(x):   # broadcast [s,L] -> [s,L,P] over priors
        return x.unsqueeze(2).to_broadcast((s, L, P))

    def bp(x):   # broadcast [s,P] -> [s,L,P] over lanes
        return x.unsqueeze(1).to_broadcast((s, L, P))

    # ======== PRE-S: everything that only needs pd_s / targets ========
    scr = b1

    def norm_score(out_t, x, tag):
        # out = 1.01 - x / max(masked max(x), 1e-30); x >= 0
        mx = pool.tile([s, 1], F32, tag=tag + "_mx")
        V.tensor_tensor(scr[:], x, bl(validf[:]), op=Alu.mult)
        V.tensor_reduce(out=mx[:], in_=scr[:], axis=AX.XY, op=Alu.max)
        V.tensor_scalar(mx[:], mx[:], 1e-30, -1.0, op0=Alu.max, op1=Alu.mult)
        V.reciprocal(mx[:], mx[:])  # = -1/max
        A.activation(out_t, x, AF.Identity, scale=mx[:], bias=c101[:])

    # start-point distance score -> ss (b6)
    dxy, sd, ss, ths = b4, b5, b6, b3
    for l in range(L):
        A.activation(sd[:, l], pd_s[:, :, 2], AF.Identity,
                     scale=IMG_H - 1.0, bias=t_y[:, l:l + 1])
        A.activation(dxy[:, l], pd_s[:, :, 3], AF.Identity,
                     scale=W_SCALE, bias=ntx[:, l:l + 1])
    A.activation(sd[:], sd[:], AF.Square)                 # dy^2
    A.activation(dxy[:], dxy[:], AF.Square)               # dx^2
    V.tensor_tensor(sd[:], sd[:], dxy[:], op=Alu.add)
    A.sqrt(sd[:], sd[:])
    norm_score(ss[:], sd[:], "s")                         # b5 free
    # theta score -> ths (b3)
    th = b4
    for l in range(L):
        A.activation(th[:, l], pd_s[:, :, 4], AF.Identity,
                     scale=1.0, bias=nth[:, l:l + 1])
    A.activation(th[:], th[:], AF.Abs)
    norm_score(ths[:], th[:], "t")                        # b4 free

    # cls cost (column 1 only): pos1 - neg1
    spc = pool.tile([s, P], F32, tag="spc")
    d01 = pool.tile([s, P], F32, tag="d01")
    V.scalar_tensor_tensor(out=d01[:], in0=pd_s[:, :, 0], scalar=-1.0,
                           in1=pd_s[:, :, 1], op0=Alu.mult, op1=Alu.add)
    A.activation(spc[:], pd_s[:, :, 1], AF.Sigmoid)
    lp = pool.tile([s, P], F32, tag="lp")
    A.activation(lp[:], spc[:], AF.Ln, bias=eps12[:])
    one_m = pool.tile([s, P], F32, tag="one_m")
    A.activation(one_m[:], spc[:], AF.Identity, scale=-1.0, bias=1.0)
    ln_ = pool.tile([s, P], F32, tag="ln_")
    A.activation(ln_[:], one_m[:], AF.Ln, bias=eps12[:])
    sq1 = pool.tile([s, P], F32, tag="sq1")
    A.activation(sq1[:], one_m[:], AF.Square)                  # (1-sp)^2
    sq0 = pool.tile([s, P], F32, tag="sq0")
    A.activation(sq0[:], spc[:], AF.Square)                    # sp^2
    clsc = pool.tile([s, P], F32, tag="clsc")
    V.scalar_tensor_tensor(out=clsc[:], in0=lp[:], scalar=-0.25, in1=sq1[:],
                           op0=Alu.mult, op1=Alu.mult)          # pos1
    V.scalar_tensor_tensor(out=ln_[:], in0=ln_[:], scalar=-0.75, in1=sq0[:],
                           op0=Alu.mult, op1=Alu.mult)          # neg1
    V.tensor_tensor(clsc[:], clsc[:], ln_[:], op=Alu.subtract)  # pos1 - neg1

    # focal pieces (independent of assignment)
    p1e = pool.tile([s, P], F32, tag="p1e")
    A.activation(p1e[:], d01[:], AF.Sigmoid)
    A.activation(p1e[:], p1e[:], AF.Identity, bias=eps8[:])
    p0e = pool.tile([s, P], F32, tag="p0e")
    A.activation(p0e[:], d01[:], AF.Sigmoid, scale=-1.0)
    A.activation(p0e[:], p0e[:], AF.Identity, bias=eps8[:])
    l1t = pool.tile([s, P], F32, tag="l1t")
    A.activation(l1t[:], p1e[:], AF.Ln)
    l0t = pool.tile([s, P], F32, tag="l0t")
    A.activation(l0t[:], p0e[:], AF.Ln)
    A.activation(p1e[:], p1e[:], AF.Identity, scale=-1.0, bias=1.0)
    A.activation(p0e[:], p0e[:], AF.Identity, scale=-1.0, bias=1.0)
    A.activation(p1e[:], p1e[:], AF.Square)               # (1-p1)^2
    A.activation(p0e[:], p0e[:], AF.Square)               # (1-p0)^2
    f1 = pool.tile([s, P], F32, tag="f1")
    V.scalar_tensor_tensor(out=f1[:], in0=l1t[:], scalar=-0.25, in1=p1e[:],
                           op0=Alu.mult, op1=Alu.mult)
    f0 = pool.tile([s, P], F32, tag="f0")
    V.scalar_tensor_tensor(out=f0[:], in0=l0t[:], scalar=-0.25, in1=p0e[:],
                           op0=Alu.mult, op1=Alu.mult)
    num_t = pool.tile([s, 1], F32, tag="num_t")
    V.tensor_reduce(out=num_t[:], in_=validf[:], axis=AX.X, op=Alu.add)
    V.tensor_scalar(num_t[:], num_t[:], 1.0, None, op0=Alu.max)
    V.reciprocal(num_t[:], num_t[:])

    # reg prep (independent of assignment): slsum -> b2
    yx = pool.tile([s, 4, P], F32, tag="yx")     # pred_yxtl, c-major
    scales = [N_STRIPS, W_SCALE, 180.0, N_STRIPS]
    for c in range(4):
        A.mul(yx[:, c], pd_s[:, :, 2 + c], scales[c])
    pstart = pool.tile([s, P], F32, tag="pstart")
    V.tensor_scalar(pstart[:], pd_s[:, :, 2], N_STRIPS, None, op0=Alu.mult)
    psi = pool.tile([s, P], mybir.dt.int32, tag="psi")
    V.tensor_copy(psi[:], pstart[:])
    V.tensor_copy(pstart[:], psi[:])        # jnp.round (RNE)
    V.tensor_scalar(pstart[:], pstart[:], 0.0, N_STRIPS, op0=Alu.max, op1=Alu.min)

    def smooth_l1(a, qq, cnd, shp):
        # in-place: a := 0.5*min(|a|,1)^2 + relu(|a|-1)  (== smooth L1)
        A.activation(a, a, AF.Abs)
        V.tensor_scalar(qq, a, 1.0, None, op0=Alu.min)        # min(|a|,1), 2x ts
        A.activation(a, a, AF.Relu, bias=neg1t[:])            # relu(|a|-1)
        V.scalar_tensor_tensor(out=qq, in0=qq, scalar=0.5, in1=qq,
                               op0=Alu.mult, op1=Alu.mult)    # 0.5*m^2
        V.tensor_tensor(a, a, qq, op=Alu.add)

    diff3 = pool.tile([s, L, 3, P], F32, tag="diff3")
    d3q = pool.tile([s, L, 3, P], F32, tag="d3q")
    V.tensor_tensor(diff3[:], yx[:, 0:3].unsqueeze(1).to_broadcast((s, L, 3, P)),
                    g3[:].unsqueeze(3).to_broadcast((s, L, 3, P)), op=Alu.subtract)
    smooth_l1(diff3[:], d3q[:], None, None)
    slsum = b2
    V.tensor_reduce(out=slsum[:], in_=diff3[:].rearrange("s l c p -> s l p c"),
                    axis=AX.X, op=Alu.add)
    tlp = b4
    V.tensor_tensor(tlp[:], bl(tsum[:]), bp(pstart[:]), op=Alu.subtract)
    V.tensor_tensor(tlp[:], bp(yx[:, 3]), tlp[:], op=Alu.subtract)  # yxtl3 - tlp
    smooth_l1(tlp[:], b5[:], None, None)
    V.tensor_tensor(slsum[:], slsum[:], tlp[:], op=Alu.add)         # b4, b5 free

    # ---------------- S' [s,L,P] -> b0 ----------------
    Sp = b0
    nchunk = P // PC
    for pc in range(nchunk):
        for l in range(L):
            v = vpool.tile([s, PC, NPTS], F32, tag="v")
            G.tensor_tensor(
                v[:], tp[:, l].unsqueeze(1).to_broadcast((s, PC, NPTS)),
                pd_xc[pc][:], op=Alu.subtract)
            V.tensor_scalar_max(v[:], v[:], -1.0)
            V.tensor_reduce(out=Sp[:, l, pc * PC:(pc + 1) * PC], in_=v[:],
                            axis=AX.X, op=Alu.add, apply_absolute_value=True)
    V.tensor_tensor(Sp[:], Sp[:], bl(n_inv[:]), op=Alu.subtract)

    # ---------------- dist score / cost / iou ----------------
    dist, ds = b5, b7
    V.tensor_tensor(dist[:], Sp[:], bl(rec_tlen[:]), op=Alu.mult)
    # dist is exactly 0 on invalid lanes (S'=0 there), so no valid-mask needed
    dmx = pool.tile([s, 1], F32, tag="d_mx")
    V.tensor_reduce(out=dmx[:], in_=dist[:], axis=AX.XY, op=Alu.max)
    V.tensor_scalar(dmx[:], dmx[:], 1e-30, -1.0, op0=Alu.max, op1=Alu.mult)
    V.reciprocal(dmx[:], dmx[:])
    A.activation(ds[:], dist[:], AF.Identity, scale=dmx[:], bias=c101[:])  # b5 free
    q = b5
    V.tensor_tensor(q[:], ds[:], ss[:], op=Alu.mult)
    V.tensor_tensor(q[:], q[:], ths[:], op=Alu.mult)     # b6, b3 free-ish
    ncost = b4
    V.scalar_tensor_tensor(out=ncost[:], in0=q[:], scalar=3.0, in1=q[:],
                           op0=Alu.mult, op1=Alu.mult)    # 3*q^2
    V.tensor_tensor(ncost[:], ncost[:], bp(clsc[:]), op=Alu.subtract)
    V.tensor_tensor(ncost[:], ncost[:], bl(validf[:]), op=Alu.mult)
    V.tensor_tensor(ncost[:], ncost[:], bl(neg_pen[:]), op=Alu.add)   # b5 free

    iou, den, rden = b7, b3, b1
    V.scalar_tensor_tensor(out=iou[:], in0=Sp[:], scalar=-1.0, in1=bl(c1[:]),
                           op0=Alu.mult, op1=Alu.add)     # c1 - S' (ds dead -> b7)
    V.tensor_tensor(den[:], Sp[:], bl(c1eps[:]), op=Alu.add)   # b0 free
    V.reciprocal_approx_accurate(rden[:], den[:], scratch=b5[:])
    V.tensor_tensor(iou[:], iou[:], rden[:], op=Alu.mult)  # b1, b3 free

    # ---------------- dynamic-k ----------------
    iw = b3
    V.scalar_tensor_tensor(out=iw[:], in0=iou[:], scalar=0.0, in1=bl(validf[:]),
                           op0=Alu.max, op1=Alu.mult)
    i8 = pool.tile([s, L, 8], F32, tag="i8")
    m8 = pool.tile([s, L, 8], F32, tag="m8")
    for l in range(L):
        V.max(out=i8[:, l], in_=iw[:, l])
        V.max(out=m8[:, l], in_=ncost[:, l])               # b3 free
    dks = pool.tile([s, L], F32, tag="dks")
    V.tensor_reduce(out=dks[:], in_=i8[:, :, 0:4], axis=AX.X, op=Alu.add)
    dkf = pool.tile([s, L], F32, tag="dkf")
    V.tensor_scalar(dkf[:], dks[:], 0.5, None, op0=Alu.subtract)
    dki = pool.tile([s, L], mybir.dt.int32, tag="dki")
    V.tensor_copy(dki[:], dkf[:])
    V.tensor_copy(dkf[:], dki[:])           # floor(dks) for non-integer dks
    V.tensor_scalar(dkf[:], dkf[:], 1.0, 0.0, op0=Alu.subtract, op1=Alu.max)
    V.tensor_scalar(dkf[:], dkf[:], 3.0, None, op0=Alu.min)     # dyn_k-1 in [0,3]
    eqm = pool.tile([s, L, L], F32, tag="eqm")
    V.tensor_tensor(eqm[:], iota4[:].unsqueeze(1).to_broadcast((s, L, L)),
                    dkf[:].unsqueeze(2).to_broadcast((s, L, L)), op=Alu.is_equal)
    V.tensor_tensor(eqm[:], eqm[:], m8[:, :, 0:4], op=Alu.mult)
    thr = pool.tile([s, L], F32, tag="thr")
    V.tensor_reduce(out=thr[:], in_=eqm[:], axis=AX.X, op=Alu.add)

    # M [s,L,P]
    M = b6
    V.tensor_tensor(M[:], ncost[:], bl(thr[:]), op=Alu.is_ge)
    V.tensor_tensor(M[:], M[:], bl(validf[:]), op=Alu.mult)
    nm_p = pool.tile([s, P], F32, tag="nm_p")
    V.tensor_reduce(out=nm_p[:], in_=M[:].rearrange("s l p -> s p l"),
                    axis=AX.X, op=Alu.add)
    multi = pool.tile([s, P], F32, tag="multi")
    V.tensor_scalar(multi[:], nm_p[:], 1.0, None, op0=Alu.is_gt)
    nmax_p = pool.tile([s, P], F32, tag="nmax_p")
    V.tensor_reduce(out=nmax_p[:], in_=ncost[:].rearrange("s l p -> s p l"),
                    axis=AX.X, op=Alu.max)
    oh = b5
    V.tensor_tensor(oh[:], ncost[:], bp(nmax_p[:]), op=Alu.is_equal)  # b4 free after
    notmulti = pool.tile([s, P], F32, tag="notmulti")
    A.activation(notmulti[:], multi[:], AF.Identity, scale=-1.0, bias=1.0)
    V.tensor_tensor(M[:, 0], M[:, 0], notmulti[:], op=Alu.mult)
    V.tensor_tensor(oh[:], oh[:], bp(multi[:]), op=Alu.mult)
    V.tensor_tensor(M[:], M[:], oh[:], op=Alu.max)        # b5 free
    n_match = pool.tile([s, 1], F32, tag="n_match")
    V.tensor_reduce(out=n_match[:], in_=M[:], axis=AX.XY, op=Alu.add)

    # ---------------- cls term ----------------
    matched = pool.tile([s, P], F32, tag="matched")
    V.tensor_reduce(out=matched[:], in_=M[:].rearrange("s l p -> s p l"),
                    axis=AX.X, op=Alu.add)
    matchedu = pool.tile([s, P], mybir.dt.uint8, tag="matchedu")
    V.tensor_scalar(matchedu[:], matched[:], 0.0, None, op0=Alu.is_gt)
    V.copy_predicated(f0[:], matchedu[:], f1[:])   # f0 := where(matched, f1, f0)
    trip = pool.tile([s, 3], F32, tag="trip")
    V.tensor_reduce(out=trip[:, 0:1], in_=f0[:], axis=AX.X, op=Alu.add)
    V.tensor_scalar(trip[:, 0:1], trip[:, 0:1], num_t[:], None, op0=Alu.mult)

    # ---------------- reg term ----------------
    V.tensor_tensor(slsum[:], slsum[:], M[:], op=Alu.mult)
    rden4 = pool.tile([s, 1], F32, tag="rden4")
    V.tensor_scalar(rden4[:], n_match[:], 4.0, 1.0, op0=Alu.mult, op1=Alu.max)
    V.reciprocal(rden4[:], rden4[:])
    V.tensor_reduce(out=trip[:, 1:2], in_=slsum[:], axis=AX.XY, op=Alu.add)
    V.tensor_scalar(trip[:, 1:2], trip[:, 1:2], rden4[:], None, op0=Alu.mult)

    # ---------------- iou term ----------------
    A.activation(iou[:], iou[:], AF.Identity, scale=-1.0, bias=1.0)
    V.tensor_tensor(iou[:], iou[:], M[:], op=Alu.mult)
    rnm = pool.tile([s, 1], F32, tag="rnm")
    V.tensor_scalar(rnm[:], n_match[:], 1.0, None, op0=Alu.max)
    V.reciprocal(rnm[:], rnm[:])
    V.tensor_reduce(out=trip[:, 2:3], in_=iou[:], axis=AX.XY, op=Alu.add)
    V.tensor_scalar(trip[:, 2:3], trip[:, 2:3], rnm[:], None, op0=Alu.mult)

    # ---------------- cross-partition sum via PE ----------------
    ones = pool.tile([s, 1], F32, tag="ones")
    V.memset(ones[:], 1.0)
    part = psum_pool.tile([1, 3], F32, tag="psum_part")
    T.matmul(part[:], ones[:], trip[:], start=True, stop=True)
    V.tensor_tensor(acc[:1, 0:3], acc[:1, 0:3], part[:], op=Alu.add)


def build():
    nc = bacc.Bacc("TRN2", target_bir_lowering=False, debug=False)
    preds_s = nc.dram_tensor("preds_s", [ROWS, P * 6], F32, kind="ExternalInput").ap()
    preds_x = nc.dram_tensor("preds_x", [ROWS, P * NPTS], F32, kind="ExternalInput").ap()
    tgts = nc.dram_tensor("tgts", [BS, L * D], F32, kind="ExternalInput").ap()
    outp = nc.dram_tensor("out", [1, 4], F32, kind="ExternalOutput").ap()

    pds3 = preds_s.rearrange("r (p d) -> r p d", d=6)
    pdx3 = preds_x.rearrange("r (p n) -> r p n", n=NPTS)
    tg3 = tgts.rearrange("r (l d) -> r l d", d=D)

    with TileContext(nc) as tc:
        from contextlib import ExitStack
        with ExitStack() as ctx:
            pool = ctx.enter_context(tc.tile_pool(name="main", bufs=1))
            vpool = ctx.enter_context(tc.tile_pool(name="vp", bufs=3))
            psum_pool = ctx.enter_context(tc.tile_pool(name="ps", bufs=2, space="PSUM"))
            acc = pool.tile([1, 4], F32, tag="acc")
            nc.vector.memset(acc[:], 0.0)
            # block 0: stages 0,1 (rows 0..127); block 1: stage 2 (rows 128..191)
            _build_block(nc, tc, pool, vpool, psum_pool,
                         pds3[0:128], pdx3[0:128],
                         [tg3, tg3], acc, 128)
            _build_block(nc, tc, pool, vpool, psum_pool,
                         pds3[128:192], pdx3[128:192],
                         [tg3], acc, 64)
            nc.sync.dma_start(outp[:], acc[:])
    nc.compile()
    return nc


_NC_CACHE = None


def _get_nc():
    global _NC_CACHE
    if _NC_CACHE is None:
        _NC_CACHE = build()
    return _NC_CACHE


def kernel(predictions, targets, seg_loss):
    nc = _get_nc()
    in_maps = []
    predictions = np.asarray(predictions)
    targets = np.asarray(targets)
    for c in range(NCORES):
        sl = slice(BS * c, BS * (c + 1))
        ps = np.ascontiguousarray(predictions[:, sl, :, 0:6]).reshape(ROWS, P * 6)
        px = np.ascontiguousarray(predictions[:, sl, :, 6:D]).reshape(ROWS, P * NPTS)
        t = np.ascontiguousarray(targets[sl]).reshape(BS, L * D)
        in_maps.append({"preds_s": ps, "preds_x": px, "tgts": t})
    res = run_bass_kernel_spmd(nc, in_maps, list(range(NCORES))).results
    tot = np.zeros(3, np.float64)
    for r in res:
        tot += r["out"][0, 0:3].astype(np.float64)
    denom = float(B * STAGES)
    loss = (2.0 * tot[0] + 0.2 * tot[1] + 2.0 * tot[2]) / denom + float(seg_loss)
    return np.float32(loss)


if __name__ == "__main__":
    build()
    print("build OK")



# revision 5
# speedup vs baseline: 1.0708x; 1.0708x over previous
"""CLRNet loss kernel for Trainium2 (8 NeuronCores, data-parallel over batch).

Contract: kernel(predictions [3,512,192,78] f32, targets [512,4,78] f32,
seg_loss scalar f32) -> scalar f32 (full loss). Sharding: batch axis split
8 ways; each core computes partial (cls, reg, iou) sums over its 64 samples
x 3 stages; host combines.

Key algebra used on-device:
  ovr.sum  = 30*T - S_px,  union.sum = 30*T + S_px, where
  S_px[p,l] = sum_n mask[l,n] * |pred_px[p,n] - t_xs[l,n]| = 799 * S'
  S'[p,l]   = sum_n min(|t'[l,n] - p_hat[p,n]|, 1) - n_invalid[l]
  (valid entries of t' = t/799 are in [0,1) and p_hat in [0,1), so
   min(|d|,1)=|d| there; invalid entries (t' ~ -125) clamp to exactly 1.)
The pairwise |d| pipeline runs in fp16: sub (tensor_tensor 2x), fused
clamp+abs (tensor_scalar abs_max/min 4x), then a contiguous-slice fold
tree 72->36->18->9 (tensor_tensor 2x) and a final 9-wide tensor_reduce
into fp32. A slice of the subtracts runs on GpSimd(Pool) to offload DVE.
"""

import numpy as np

import concourse.bacc as bacc
import concourse.mybir as mybir
from concourse.tile import TileContext
from concourse.bass_utils import run_bass_kernel_spmd

F32 = mybir.dt.float32
F16 = mybir.dt.float16
Alu = mybir.AluOpType
AF = mybir.ActivationFunctionType
AX = mybir.AxisListType

STAGES, B, P, D = 3, 512, 192, 78
NPTS = 72
L = 4
NCORES = 8
BS = B // NCORES            # 64 batch samples per core
ROWS = STAGES * BS          # 192 virtual samples per core
IMG_W, IMG_H = 800.0, 320.0
N_STRIPS = float(NPTS - 1)
W_SCALE = IMG_W - 1.0       # 799
BIG = 1.0e30
PC = 64                     # prior chunk for the heavy S pipeline
POOL_SUBS = 4               # of the 12 (l,pc) units per block, how many
                            # subtracts run on GpSimd instead of DVE


def _build_block(nc, tc, pool, vpool, psum_pool, pd_small_dram, pd_xs_dram,
                 tm_dram, tp16_dram, acc, s):
    """Emit one block of `s` samples (s <= 128 partitions)."""
    V, G, A, T = nc.vector, nc.gpsimd, nc.scalar, nc.tensor

    # ---------------- loads ----------------
    pd_s = pool.tile([s, P, 6], F32, tag="pd_small")
    nc.sync.dma_start(pd_s[:], pd_small_dram)
    NTM = 16
    tm = pool.tile([s, NTM, L], F32, tag="tm")   # host tmeta, channel-major
    tp16 = pool.tile([s, L, NPTS], F16, tag="tp16")
    r0 = 0
    for tmsrc, tpsrc in zip(tm_dram, tp16_dram):
        n = tmsrc.shape[0]
        nc.sync.dma_start(tm[r0:r0 + n], tmsrc)
        nc.sync.dma_start(tp16[r0:r0 + n], tpsrc)
        r0 += n
    pd_xc = []
    for pc in range(P // PC):
        t_ = pool.tile([s, PC, NPTS], F16, tag="pd_xc", bufs=4, name=f"pd_xc{pc}")
        nc.sync.dma_start(t_[:], pd_xs_dram[:, pc * PC:(pc + 1) * PC])
        pd_xc.append(t_)

    # host-provided per-lane scalars (channels of tm)
    validf = tm[:, 0]
    n_inv = tm[:, 1]
    rec_tlen = tm[:, 2]
    c1 = tm[:, 3]            # 30*T/799
    c1eps = tm[:, 4]
    t_y = tm[:, 5]           # -(IMG_H-1)*tg2
    ntx = tm[:, 6]           # -tg3
    nth = tm[:, 7]           # -tg4
    tsum = tm[:, 8]          # round(tg2*71) + tg5
    g30 = tm[:, 9]           # tg2*71
    g31 = tm[:, 10]          # tg3
    g32 = tm[:, 11]          # tg4*180
    neg_pen = tm[:, 12]      # 0 valid / -BIG invalid

    # big reusable [s, L, P] buffers
    bufs = [pool.tile([s, L, P], F32, tag=f"big{i}", name=f"big{i}") for i in range(8)]
    b0, b1, b2, b3, b4, b5, b6, b7 = bufs

    iota4 = pool.tile([s, L], F32, tag="iota4")
    for j in range(L):
        V.memset(iota4[:, j:j + 1], float(j))
    eps12 = pool.tile([s, 1], F32, tag="eps12")
    V.memset(eps12[:], 1e-12)
    eps8 = pool.tile([s, 1], F32, tag="eps8")
    V.memset(eps8[:], 1e-8)
    c101 = pool.tile([s, 1], F32, tag="c101")
    V.memset(c101[:], 1.01)
    neg1t = pool.tile([s, 1], F32, tag="neg1t")
    V.memset(neg1t[:], -1.0)

    def bl(x):   # broadcast [s,L] -> [s,L,P] over priors
        return x.unsqueeze(2).to_broadcast((s, L, P))

    def bp(x):   # broadcast [s,P] -> [s,L,P] over lanes
        return x.unsqueeze(1).to_broadcast((s, L, P))

    # ======== PRE-S: everything that only needs pd_s / targets ========
    scr = b1

    def norm_score(out_t, x, tag):
        # out = 1.01 - x / max(masked max(x), 1e-30); x >= 0
        mx = pool.tile([s, 1], F32, tag=tag + "_mx")
        V.tensor_tensor(scr[:], x, bl(validf[:]), op=Alu.mult)
        V.tensor_reduce(out=mx[:], in_=scr[:], axis=AX.XY, op=Alu.max)
        V.tensor_scalar(mx[:], mx[:], 1e-30, -1.0, op0=Alu.max, op1=Alu.mult)
        V.reciprocal(mx[:], mx[:])  # = -1/max
        A.activation(out_t, x, AF.Identity, scale=mx[:], bias=c101[:])

    # start-point distance score -> ss (b6)
    dxy, sd, ss, ths = b4, b5, b6, b3
    for l in range(L):
        A.activation(sd[:, l], pd_s[:, :, 2], AF.Identity,
                     scale=IMG_H - 1.0, bias=t_y[:, l:l + 1])
        A.activation(dxy[:, l], pd_s[:, :, 3], AF.Identity,
                     scale=W_SCALE, bias=ntx[:, l:l + 1])
    A.activation(sd[:], sd[:], AF.Square)                 # dy^2
    A.activation(dxy[:], dxy[:], AF.Square)               # dx^2
    V.tensor_tensor(sd[:], sd[:], dxy[:], op=Alu.add)
    A.sqrt(sd[:], sd[:])
    norm_score(ss[:], sd[:], "s")                         # b5 free
    # theta score -> ths (b3)
    th = b4
    for l in range(L):
        A.activation(th[:, l], pd_s[:, :, 4], AF.Identity,
                     scale=1.0, bias=nth[:, l:l + 1])
    A.activation(th[:], th[:], AF.Abs)
    norm_score(ths[:], th[:], "t")                        # b4 free

    # cls cost (column 1 only): pos1 - neg1
    spc = pool.tile([s, P], F32, tag="spc")
    d01 = pool.tile([s, P], F32, tag="d01")
    V.scalar_tensor_tensor(out=d01[:], in0=pd_s[:, :, 0], scalar=-1.0,
                           in1=pd_s[:, :, 1], op0=Alu.mult, op1=Alu.add)
    A.activation(spc[:], pd_s[:, :, 1], AF.Sigmoid)
    lp = pool.tile([s, P], F32, tag="lp")
    A.activation(lp[:], spc[:], AF.Ln, bias=eps12[:])
    one_m = pool.tile([s, P], F32, tag="one_m")
    A.activation(one_m[:], spc[:], AF.Identity, scale=-1.0, bias=1.0)
    ln_ = pool.tile([s, P], F32, tag="ln_")
    A.activation(ln_[:], one_m[:], AF.Ln, bias=eps12[:])
    sq1 = pool.tile([s, P], F32, tag="sq1")
    A.activation(sq1[:], one_m[:], AF.Square)                  # (1-sp)^2
    sq0 = pool.tile([s, P], F32, tag="sq0")
    A.activation(sq0[:], spc[:], AF.Square)                    # sp^2
    clsc = pool.tile([s, P], F32, tag="clsc")
    V.scalar_tensor_tensor(out=clsc[:], in0=lp[:], scalar=-0.25, in1=sq1[:],
                           op0=Alu.mult, op1=Alu.mult)          # pos1
    V.scalar_tensor_tensor(out=ln_[:], in0=ln_[:], scalar=-0.75, in1=sq0[:],
                           op0=Alu.mult, op1=Alu.mult)          # neg1
    V.tensor_tensor(clsc[:], clsc[:], ln_[:], op=Alu.subtract)  # pos1 - neg1

    # focal pieces (independent of assignment)
    p1e = pool.tile([s, P], F32, tag="p1e")
    A.activation(p1e[:], d01[:], AF.Sigmoid)
    A.activation(p1e[:], p1e[:], AF.Identity, bias=eps8[:])
    p0e = pool.tile([s, P], F32, tag="p0e")
    A.activation(p0e[:], d01[:], AF.Sigmoid, scale=-1.0)
    A.activation(p0e[:], p0e[:], AF.Identity, bias=eps8[:])
    l1t = pool.tile([s, P], F32, tag="l1t")
    A.activation(l1t[:], p1e[:], AF.Ln)
    l0t = pool.tile([s, P], F32, tag="l0t")
    A.activation(l0t[:], p0e[:], AF.Ln)
    A.activation(p1e[:], p1e[:], AF.Identity, scale=-1.0, bias=1.0)
    A.activation(p0e[:], p0e[:], AF.Identity, scale=-1.0, bias=1.0)
    A.activation(p1e[:], p1e[:], AF.Square)               # (1-p1)^2
    A.activation(p0e[:], p0e[:], AF.Square)               # (1-p0)^2
    f1 = pool.tile([s, P], F32, tag="f1")
    V.scalar_tensor_tensor(out=f1[:], in0=l1t[:], scalar=-0.25, in1=p1e[:],
                           op0=Alu.mult, op1=Alu.mult)
    f0 = pool.tile([s, P], F32, tag="f0")
    V.scalar_tensor_tensor(out=f0[:], in0=l0t[:], scalar=-0.25, in1=p0e[:],
                           op0=Alu.mult, op1=Alu.mult)
    num_t = pool.tile([s, 1], F32, tag="num_t")
    V.tensor_reduce(out=num_t[:], in_=validf[:], axis=AX.X, op=Alu.add)
    V.tensor_scalar(num_t[:], num_t[:], 1.0, None, op0=Alu.max)
    V.reciprocal(num_t[:], num_t[:])

    # reg prep (independent of assignment): slsum -> b2
    yx = pool.tile([s, 4, P], F32, tag="yx")     # pred_yxtl, c-major
    scales = [N_STRIPS, W_SCALE, 180.0, N_STRIPS]
    for c in range(4):
        A.mul(yx[:, c], pd_s[:, :, 2 + c], scales[c])
    pstart = pool.tile([s, P], F32, tag="pstart")
    V.tensor_scalar(pstart[:], pd_s[:, :, 2], N_STRIPS, None, op0=Alu.mult)
    psi = pool.tile([s, P], mybir.dt.int32, tag="psi")
    V.tensor_copy(psi[:], pstart[:])
    V.tensor_copy(pstart[:], psi[:])        # jnp.round (RNE)
    V.tensor_scalar(pstart[:], pstart[:], 0.0, N_STRIPS, op0=Alu.max, op1=Alu.min)

    def smooth_l1(a, qq):
        # in-place: a := 0.5*min(|a|,1)^2 + relu(|a|-1)  (== smooth L1)
        A.activation(a, a, AF.Abs)
        V.tensor_scalar(qq, a, 1.0, None, op0=Alu.min)        # min(|a|,1)
        A.activation(a, a, AF.Relu, bias=neg1t[:])            # relu(|a|-1)
        V.scalar_tensor_tensor(out=qq, in0=qq, scalar=0.5, in1=qq,
                               op0=Alu.mult, op1=Alu.mult)    # 0.5*m^2
        V.tensor_tensor(a, a, qq, op=Alu.add)

    g3 = pool.tile([s, L, 3], F32, tag="g3")
    V.tensor_copy(g3[:, :, 0], g30[:])
    V.tensor_copy(g3[:, :, 1], g31[:])
    V.tensor_copy(g3[:, :, 2], g32[:])
    diff3 = pool.tile([s, L, 3, P], F32, tag="diff3")
    d3q = pool.tile([s, L, 3, P], F32, tag="d3q")
    V.tensor_tensor(diff3[:], yx[:, 0:3].unsqueeze(1).to_broadcast((s, L, 3, P)),
                    g3[:].unsqueeze(3).to_broadcast((s, L, 3, P)), op=Alu.subtract)
    smooth_l1(diff3[:], d3q[:])
    slsum = b2
    V.tensor_reduce(out=slsum[:], in_=diff3[:].rearrange("s l c p -> s l p c"),
                    axis=AX.X, op=Alu.add)
    tlp = b4
    V.tensor_tensor(tlp[:], bl(tsum[:]), bp(pstart[:]), op=Alu.subtract)
    V.tensor_tensor(tlp[:], bp(yx[:, 3]), tlp[:], op=Alu.subtract)  # yxtl3 - tlp
    smooth_l1(tlp[:], b5[:])
    V.tensor_tensor(slsum[:], slsum[:], tlp[:], op=Alu.add)         # b4, b5 free

    # ---------------- S' [s,L,P] -> b0  (fp16 pipeline) ----------------
    # S'raw = sum_n |clamp(t'-p, -1, 1)|  (valid |d|<1 so = |d|; invalid
    # t' ~ -125 clamps to -1 -> 1);  S' = S'raw - n_inv.
    # Per unit: sub (DVE tt16 2x, some on Pool), clamp (DVE ts16 4x),
    # abs (Act), fold tree 72->36->18->9 (DVE tt16 2x), 9-wide reduce.
    Sp = b0
    nchunk = P // PC
    ui = 0
    for pc in range(nchunk):
        for l in range(L):
            v = vpool.tile([s, PC, NPTS], F16, tag="v")
            e = vpool.tile([s, PC, NPTS], F16, tag="e")
            q = vpool.tile([s, PC, NPTS], F16, tag="q")
            eng = G if (ui % 12) < POOL_SUBS else V
            eng.tensor_tensor(
                v[:], tp16[:, l].unsqueeze(1).to_broadcast((s, PC, NPTS)),
                pd_xc[pc][:], op=Alu.subtract)
            V.tensor_scalar(e[:], v[:], -1.0, 1.0, op0=Alu.max, op1=Alu.min)
            A.activation(q[:], e[:], AF.Abs)
            V.tensor_tensor(e[:, :, 0:36], q[:, :, 0:36], q[:, :, 36:72],
                            op=Alu.add)
            V.tensor_tensor(v[:, :, 0:18], e[:, :, 0:18], e[:, :, 18:36],
                            op=Alu.add)
            V.tensor_tensor(e[:, :, 0:9], v[:, :, 0:9], v[:, :, 9:18],
                            op=Alu.add)
            V.tensor_reduce(out=Sp[:, l, pc * PC:(pc + 1) * PC],
                            in_=e[:, :, 0:9], axis=AX.X, op=Alu.add)
            ui += 1
    V.tensor_tensor(Sp[:], Sp[:], bl(n_inv[:]), op=Alu.subtract)

    # ---------------- dist score / cost / iou ----------------
    dist, ds = b5, b7
    V.tensor_tensor(dist[:], Sp[:], bl(rec_tlen[:]), op=Alu.mult)
    # dist is exactly 0 on invalid lanes (S'=0 there), so no valid-mask needed
    dmx = pool.tile([s, 1], F32, tag="d_mx")
    V.tensor_reduce(out=dmx[:], in_=dist[:], axis=AX.XY, op=Alu.max)
    V.tensor_scalar(dmx[:], dmx[:], 1e-30, -1.0, op0=Alu.max, op1=Alu.mult)
    V.reciprocal(dmx[:], dmx[:])
    A.activation(ds[:], dist[:], AF.Identity, scale=dmx[:], bias=c101[:])  # b5 free
    q = b5
    V.tensor_tensor(q[:], ds[:], ss[:], op=Alu.mult)
    V.tensor_tensor(q[:], q[:], ths[:], op=Alu.mult)     # b6, b3 free-ish
    ncost = b4
    V.scalar_tensor_tensor(out=ncost[:], in0=q[:], scalar=3.0, in1=q[:],
                           op0=Alu.mult, op1=Alu.mult)    # 3*q^2
    V.tensor_tensor(ncost[:], ncost[:], bp(clsc[:]), op=Alu.subtract)
    V.tensor_tensor(ncost[:], ncost[:], bl(validf[:]), op=Alu.mult)
    V.tensor_tensor(ncost[:], ncost[:], bl(neg_pen[:]), op=Alu.add)   # b5 free

    iou, den, rden = b7, b3, b1
    V.scalar_tensor_tensor(out=iou[:], in0=Sp[:], scalar=-1.0, in1=bl(c1[:]),
                           op0=Alu.mult, op1=Alu.add)     # c1 - S'
    V.tensor_tensor(den[:], Sp[:], bl(c1eps[:]), op=Alu.add)   # b0 free
    V.reciprocal_approx_accurate(rden[:], den[:], scratch=b5[:])
    V.tensor_tensor(iou[:], iou[:], rden[:], op=Alu.mult)  # b1, b3 free

    # ---------------- dynamic-k ----------------
    iw = b3
    V.scalar_tensor_tensor(out=iw[:], in0=iou[:], scalar=0.0, in1=bl(validf[:]),
                           op0=Alu.max, op1=Alu.mult)
    i8 = pool.tile([s, L, 8], F32, tag="i8")
    m8 = pool.tile([s, L, 8], F32, tag="m8")
    for l in range(L):
        V.max(out=i8[:, l], in_=iw[:, l])
        V.max(out=m8[:, l], in_=ncost[:, l])               # b3 free
    dks = pool.tile([s, L], F32, tag="dks")
    V.tensor_reduce(out=dks[:], in_=i8[:, :, 0:4], axis=AX.X, op=Alu.add)
    dkf = pool.tile([s, L], F32, tag="dkf")
    V.tensor_scalar(dkf[:], dks[:], 0.5, None, op0=Alu.subtract)
    dki = pool.tile([s, L], mybir.dt.int32, tag="dki")
    V.tensor_copy(dki[:], dkf[:])
    V.tensor_copy(dkf[:], dki[:])           # floor(dks) for non-integer dks
    V.tensor_scalar(dkf[:], dkf[:], 1.0, 0.0, op0=Alu.subtract, op1=Alu.max)
    V.tensor_scalar(dkf[:], dkf[:], 3.0, None, op0=Alu.min)     # dyn_k-1 in [0,3]
    eqm = pool.tile([s, L, L], F32, tag="eqm")
    V.tensor_tensor(eqm[:], iota4[:].unsqueeze(1).to_broadcast((s, L, L)),
                    dkf[:].unsqueeze(2).to_broadcast((s, L, L)), op=Alu.is_equal)
    V.tensor_tensor(eqm[:], eqm[:], m8[:, :, 0:4], op=Alu.mult)
    thr = pool.tile([s, L], F32, tag="thr")
    V.tensor_reduce(out=thr[:], in_=eqm[:], axis=AX.X, op=Alu.add)

    # M [s,L,P]
    M = b6
    V.tensor_tensor(M[:], ncost[:], bl(thr[:]), op=Alu.is_ge)
    V.tensor_tensor(M[:], M[:], bl(validf[:]), op=Alu.mult)
    nm_p = pool.tile([s, P], F32, tag="nm_p")
    V.tensor_reduce(out=nm_p[:], in_=M[:].rearrange("s l p -> s p l"),
                    axis=AX.X, op=Alu.add)
    multi = pool.tile([s, P], F32, tag="multi")
    V.tensor_scalar(multi[:], nm_p[:], 1.0, None, op0=Alu.is_gt)
    nmax_p = pool.tile([s, P], F32, tag="nmax_p")
    V.tensor_reduce(out=nmax_p[:], in_=ncost[:].rearrange("s l p -> s p l"),
                    axis=AX.X, op=Alu.max)
    oh = b5
    V.tensor_tensor(oh[:], ncost[:], bp(nmax_p[:]), op=Alu.is_equal)
    notmulti = pool.tile([s, P], F32, tag="notmulti")
    A.activation(notmulti[:], multi[:], AF.Identity, scale=-1.0, bias=1.0)
    V.tensor_tensor(M[:, 0], M[:, 0], notmulti[:], op=Alu.mult)
    V.tensor_tensor(oh[:], oh[:], bp(multi[:]), op=Alu.mult)
    V.tensor_tensor(M[:], M[:], oh[:], op=Alu.max)        # b5 free
    n_match = pool.tile([s, 1], F32, tag="n_match")
    V.tensor_reduce(out=n_match[:], in_=M[:], axis=AX.XY, op=Alu.add)

    # ---------------- cls term ----------------
    matched = pool.tile([s, P], F32, tag="matched")
    V.tensor_reduce(out=matched[:], in_=M[:].rearrange("s l p -> s p l"),
                    axis=AX.X, op=Alu.add)
    matchedu = pool.tile([s, P], mybir.dt.uint8, tag="matchedu")
    V.tensor_scalar(matchedu[:], matched[:], 0.0, None, op0=Alu.is_gt)
    V.copy_predicated(f0[:], matchedu[:], f1[:])   # f0 := where(matched, f1, f0)
    trip = pool.tile([s, 3], F32, tag="trip")
    V.tensor_reduce(out=trip[:, 0:1], in_=f0[:], axis=AX.X, op=Alu.add)
    V.tensor_scalar(trip[:, 0:1], trip[:, 0:1], num_t[:], None, op0=Alu.mult)

    # ---------------- reg term ----------------
    V.tensor_tensor(slsum[:], slsum[:], M[:], op=Alu.mult)
    rden4 = pool.tile([s, 1], F32, tag="rden4")
    V.tensor_scalar(rden4[:], n_match[:], 4.0, 1.0, op0=Alu.mult, op1=Alu.max)
    V.reciprocal(rden4[:], rden4[:])
    V.tensor_reduce(out=trip[:, 1:2], in_=slsum[:], axis=AX.XY, op=Alu.add)
    V.tensor_scalar(trip[:, 1:2], trip[:, 1:2], rden4[:], None, op0=Alu.mult)

    # ---------------- iou term ----------------
    A.activation(iou[:], iou[:], AF.Identity, scale=-1.0, bias=1.0)
    V.tensor_tensor(iou[:], iou[:], M[:], op=Alu.mult)
    rnm = pool.tile([s, 1], F32, tag="rnm")
    V.tensor_scalar(rnm[:], n_match[:], 1.0, None, op0=Alu.max)
    V.reciprocal(rnm[:], rnm[:])
    V.tensor_reduce(out=trip[:, 2:3], in_=iou[:], axis=AX.XY, op=Alu.add)
    V.tensor_scalar(trip[:, 2:3], trip[:, 2:3], rnm[:], None, op0=Alu.mult)

    # ---------------- cross-partition sum via PE ----------------
    ones = pool.tile([s, 1], F32, tag="ones")
    V.memset(ones[:], 1.0)
    part = psum_pool.tile([1, 3], F32, tag="psum_part")
    T.matmul(part[:], ones[:], trip[:], start=True, stop=True)
    V.tensor_tensor(acc[:1, 0:3], acc[:1, 0:3], part[:], op=Alu.add)


def build():
    nc = bacc.Bacc("TRN2", target_bir_lowering=False, debug=False)
    preds_s = nc.dram_tensor("preds_s", [ROWS, P * 6], F32, kind="ExternalInput").ap()
    preds_x = nc.dram_tensor("preds_x", [ROWS, P * NPTS], F16, kind="ExternalInput").ap()
    tmeta = nc.dram_tensor("tmeta", [BS, 16 * L], F32, kind="ExternalInput").ap()
    tp16d = nc.dram_tensor("tp16", [BS, L * NPTS], F16, kind="ExternalInput").ap()
    outp = nc.dram_tensor("out", [1, 4], F32, kind="ExternalOutput").ap()

    pds3 = preds_s.rearrange("r (p d) -> r p d", d=6)
    pdx3 = preds_x.rearrange("r (p n) -> r p n", n=NPTS)
    tm3 = tmeta.rearrange("r (c l) -> r c l", l=L)
    tp3 = tp16d.rearrange("r (l n) -> r l n", n=NPTS)

    with TileContext(nc) as tc:
        from contextlib import ExitStack
        with ExitStack() as ctx:
            pool = ctx.enter_context(tc.tile_pool(name="main", bufs=1))
            vpool = ctx.enter_context(tc.tile_pool(name="vp", bufs=3))
            psum_pool = ctx.enter_context(tc.tile_pool(name="ps", bufs=2, space="PSUM"))
            acc = pool.tile([1, 4], F32, tag="acc")
            nc.vector.memset(acc[:], 0.0)
            # block 0: stages 0,1 (rows 0..127); block 1: stage 2 (rows 128..191)
            _build_block(nc, tc, pool, vpool, psum_pool,
                         pds3[0:128], pdx3[0:128],
                         [tm3, tm3], [tp3, tp3], acc, 128)
            _build_block(nc, tc, pool, vpool, psum_pool,
                         pds3[128:192], pdx3[128:192],
                         [tm3], [tp3], acc, 64)
            nc.sync.dma_start(outp[:], acc[:])
    nc.compile()
    return nc


_NC_CACHE = None


def _get_nc():
    global _NC_CACHE
    if _NC_CACHE is None:
        _NC_CACHE = build()
    return _NC_CACHE


def _prep_inputs(predictions, targets):
    """Host-side marshalling: shard, split fields, fp16-convert, and
    precompute per-lane target scalars (tmeta) exactly as the reference
    derives them from `targets`."""
    predictions = np.asarray(predictions, dtype=np.float32)
    targets = np.asarray(targets, dtype=np.float32)

    t_xs = targets[:, :, 6:]                                   # [B,L,N]
    inv = (t_xs < 0) | (t_xs >= IMG_W)
    n_inv = inv.sum(-1).astype(np.float32)                     # [B,L]
    t_len = float(NPTS) - n_inv
    validf = (targets[:, :, 1] == 1.0).astype(np.float32)
    tp = (t_xs / W_SCALE).astype(np.float16)                   # t' fp16

    tm = np.zeros((B, 16, L), np.float32)
    tm[:, 0] = validf
    tm[:, 1] = n_inv
    tm[:, 2] = 1.0 / (t_len + 1e-9)
    tm[:, 3] = 30.0 / W_SCALE * t_len                          # c1
    tm[:, 4] = tm[:, 3] + 1e-9 / W_SCALE                       # c1eps
    tm[:, 5] = -(IMG_H - 1.0) * targets[:, :, 2]               # t_y
    tm[:, 6] = -targets[:, :, 3]                               # ntx
    tm[:, 7] = -targets[:, :, 4]                               # nth
    tstart = np.round(targets[:, :, 2] * N_STRIPS)
    tm[:, 8] = tstart + targets[:, :, 5]                       # tsum
    tm[:, 9] = targets[:, :, 2] * N_STRIPS                     # g3_0
    tm[:, 10] = targets[:, :, 3]                               # g3_1
    tm[:, 11] = targets[:, :, 4] * 180.0                       # g3_2
    tm[:, 12] = np.where(validf > 0, 0.0, -BIG)                # neg_pen

    in_maps = []
    for c in range(NCORES):
        sl = slice(BS * c, BS * (c + 1))
        ps = np.ascontiguousarray(predictions[:, sl, :, 0:6]).reshape(ROWS, P * 6)
        px = np.ascontiguousarray(
            predictions[:, sl, :, 6:D].astype(np.float16)).reshape(ROWS, P * NPTS)
        tmc = np.ascontiguousarray(tm[sl]).reshape(BS, 16 * L)
        tpc = np.ascontiguousarray(tp[sl]).reshape(BS, L * NPTS)
        in_maps.append({"preds_s": ps, "preds_x": px, "tmeta": tmc, "tp16": tpc})
    return in_maps


def kernel(predictions, targets, seg_loss):
    nc = _get_nc()
    in_maps = _prep_inputs(predictions, targets)
    res = run_bass_kernel_spmd(nc, in_maps, list(range(NCORES))).results
    tot = np.zeros(3, np.float64)
    for r in res:
        tot += r["out"][0, 0:3].astype(np.float64)
    denom = float(B * STAGES)
    loss = (2.0 * tot[0] + 0.2 * tot[1] + 2.0 * tot[2]) / denom + float(seg_loss)
    return np.float32(loss)


if __name__ == "__main__":
    build()
    print("build OK")
